# revision 1
# baseline (speedup 1.0000x reference)
"""Trainium2 Bass kernel for CustomGATConv (dense masked GAT attention).

Strategy (8-core SPMD, row-sharded attention):
  - Each core owns 512 destination rows i of the [4096, 4096, 8] attention
    tensor.  Inputs are node-rotated per core so that the identical program
    always works on rows [0:512) of its own rotated node order.
  - h = x @ W is computed on every core (replicated, cheap on PE).
  - Per (row-block, head): z[j, i] = e_src[i] + e_dst[j] + (-200 if masked)
    is built entirely in PSUM by three tiny matmuls (rank-1/2 outer products
    plus an identity-weighted mask inject), so the ScalarEngine only runs
    two activation passes: Prelu(alpha=0.2) then Exp.  exp(-200ish) == 0
    implements the mask.
  - alpha @ h and the softmax denominator come from one accumulated matmul
    against h augmented with a ones column ([K=128 j, 65]).
  - Normalization: PE-transpose of the [65, 512] accumulator, then a DVE
    reciprocal + per-partition scalar multiply.
"""

import re

import numpy as np
import ml_dtypes

import bass_rust as br
import concourse.bass as bass
import concourse.tile as tile
from concourse import mybir
from concourse.bass_utils import run_bass_kernel_spmd

N = 4096
IN = 256
H = 8
F = 64
NCORES = 8
R = N // NCORES          # 512 destination rows per core
JT = N // 128            # 32 j-tiles
KC = IN // 128           # 2 contraction chunks for x @ W
NEG = -200.0             # additive mask value
FP = mybir.dt.float32
BF = mybir.dt.bfloat16
F16 = mybir.dt.float16


class _TileContext(tile.TileContext):
    """TileContext whose final drain splits its semaphore waits one per
    instruction — this walrus's CTRL_NO encoding only fits one sync wait."""

    def _drain_and_barrier(self, tick_clock, wait_clock):
        gc = tick_clock.global_clock
        vals = list(map(int, re.findall(r"\d+", repr(gc))))
        nonzero = [(i, t) for i, t in enumerate(vals) if t > 0]
        prev = br.VectorClock()
        partial = br.VectorClock()
        for i, t in nonzero:
            partial.require_at_least(i, t)
            inst = self.nc.sync.drain().ins
            wait_clock.add_sem_waits(
                inst,
                br.ScopedClock({None: partial.copy()}),
                br.ScopedClock({None: prev.copy()}),
            )
            prev = partial.copy()
        drain_inst = self.nc.sync.drain().ins
        wait_clock.add_sem_waits(
            drain_inst,
            br.ScopedClock({None: gc}),
            br.ScopedClock({None: prev.copy()}),
        )
        self.nc.all_engine_barrier()
        popped = self.nc._tile_sem_poison_stack.pop()
        assert popped is self._sem_poison
        self.nc.clear_and_free_semaphores(list(self.sems.allocated().values()))
        self.nc.all_engine_barrier()


def _split_excess_waits(nc, cap_compute=1, cap_nop=1):
    """This walrus encodes at most ~2 sync waits per compute instruction and
    1 per CTRL_NO (nop/drain).  Move excess waits onto injected same-engine
    nops placed immediately before the over-subscribed instruction."""
    n_split = 0
    for fn in nc.m.functions:
        for bb in fn.blocks:
            lst = bb.instructions
            i = 0
            while i < len(lst):
                inst = lst[i]
                si = inst.sync_info
                waits = list(si.on_wait) if si is not None else []
                is_ctrl = isinstance(inst, (mybir.InstNoOp, mybir.InstDrain))
                cap = cap_nop if is_ctrl else cap_compute
                if len(waits) > cap:
                    excess, keep = waits[:-cap], waits[-cap:]
                    for w in excess:
                        nop = mybir.InstNoOp(name=f"waitsplit-{nc.next_id()}")
                        nop.engine = inst.engine
                        nop.sync_info = br.SyncInfo(on_wait=[w], on_update=[])
                        lst.insert(i, nop)
                        i += 1
                        n_split += 1
                    inst.sync_info = br.SyncInfo(
                        on_wait=keep, on_update=list(si.on_update)
                    )
                i += 1
    return n_split


def _build_program(repeat=1):
    nc = bass.Bass("TRN2", target_bir_lowering=False, debug=False)
    ap = {}
    ap["xT"] = nc.dram_tensor("xT", [IN, N], FP, kind="ExternalInput").ap()
    ap["w"] = nc.dram_tensor("w", [IN, H * F], FP, kind="ExternalInput").ap()
    ap["wa"] = nc.dram_tensor("wa", [IN, 2 * H], FP, kind="ExternalInput").ap()
    ap["maskadd"] = nc.dram_tensor("maskadd", [N, R], BF, kind="ExternalInput").ap()
    ap["identb"] = nc.dram_tensor("identb", [128, 128], BF, kind="ExternalInput").ap()
    ap["identf"] = nc.dram_tensor("identf", [128, 128], FP, kind="ExternalInput").ap()
    ap["onesh"] = nc.dram_tensor("onesh", [1, H, R], FP, kind="ExternalInput").ap()
    out_ap = nc.dram_tensor("out", [R, H * F], FP, kind="ExternalOutput").ap()

    with _TileContext(nc) as tc:
        _emit(tc, nc, ap, out_ap, repeat)
    _split_excess_waits(nc)
    return nc


def _emit(tc, nc, ap, out_ap, repeat):
    from contextlib import ExitStack

    Act = mybir.ActivationFunctionType
    with ExitStack() as ctx:
        singles = ctx.enter_context(tc.tile_pool(name="singles", bufs=1))

        # ---- persistent tiles ----
        mask_sb = singles.tile([128, JT, R], BF)
        nc.sync.dma_start(mask_sb[:], ap["maskadd"].rearrange("(jt p) i -> p jt i", p=128))
        identb_sb = singles.tile([128, 128], BF)
        nc.sync.dma_start(identb_sb[:], ap["identb"])
        identf_sb = singles.tile([128, 128], FP)
        nc.sync.dma_start(identf_sb[:], ap["identf"])

        haug_sb = singles.tile([128, JT, H, F + 1], FP)
        nc.vector.memset(haug_sb[:, :, :, F:F + 1], 1.0)
        esd_sb = singles.tile([16, N], FP)
        # zsrc2[{0,32}, h, :] = e_src row of head h; zsrc2[{1,33}, h, :] =
        # ones — the K=2 z-matmul rhs, replicated at partition bases 0 and
        # 32 because lhsT and rhs must share their base partition.
        zsrc2 = singles.tile([34, H, R], FP)
        nc.sync.dma_start(out=zsrc2[1:2, :, :], in_=ap["onesh"])
        nc.sync.dma_start(out=zsrc2[33:34, :, :], in_=ap["onesh"])
        # dst_quad[:, s, :]: lhsT pairs (manual ping-pong on s).  Rows 0/32
        # are all-ones; rows 1/33 receive the two e_dst row slices by DMA
        # each iteration (PE lhsT base partition must be 0/32/64).
        dst_quad = singles.tile([34, 4, 128], FP)
        nc.vector.memset(dst_quad[0:1, :, :], 1.0)
        nc.vector.memset(dst_quad[32:33, :, :], 1.0)
        outsb = singles.tile([128, 4, H * F], FP)

        # ---- stage B: h = x @ W (node-major), esdT = (x @ WA)^T ----
        with tc.tile_pool(name="bigin", bufs=1) as bigin, \
             tc.tile_pool(name="hpsum", bufs=2, space="PSUM") as hpsum:
            xT_sb = bigin.tile([128, KC, N], FP)
            nc.sync.dma_start(xT_sb[:], ap["xT"].rearrange("(k p) n -> p k n", p=128))
            w_sb = bigin.tile([128, KC, H * F], FP)
            nc.sync.dma_start(w_sb[:], ap["w"].rearrange("(k p) f -> p k f", p=128))
            wa_sb = bigin.tile([128, KC, 2 * H], FP)
            nc.sync.dma_start(wa_sb[:], ap["wa"].rearrange("(k p) f -> p k f", p=128))

            for m in range(JT):
                ph = hpsum.tile([128, H * F], FP, tag="ph")
                for k in range(KC):
                    nc.tensor.matmul(
                        ph[:],
                        lhsT=xT_sb[:, k, m * 128:(m + 1) * 128],
                        rhs=w_sb[:, k, :],
                        start=(k == 0),
                        stop=(k == KC - 1),
                    )
                nc.vector.tensor_copy(
                    out=haug_sb[:, m, :, 0:F],
                    in_=ph[:].rearrange("p (h f) -> p h f", h=H),
                )
            for q in range(8):
                pe = hpsum.tile([16, R], FP, tag="pe")
                for k in range(KC):
                    nc.tensor.matmul(
                        pe[:],
                        lhsT=wa_sb[:, k, :],
                        rhs=xT_sb[:, k, q * R:(q + 1) * R],
                        start=(k == 0),
                        stop=(k == KC - 1),
                    )
                nc.vector.tensor_copy(out=esd_sb[:, q * R:(q + 1) * R], in_=pe[:])

        # stage all e_src rows (partitions 0..7) into row 0 of zsrc2 in one
        # SBUF->SBUF DMA.  Compute engines can only address partition bases
        # {0,32,64}; DMA has no such restriction.
        nc.gpsimd.dma_start(
            out=zsrc2[0:1, :, :],
            in_=esd_sb[0:8, 0:R],
        )
        nc.gpsimd.dma_start(
            out=zsrc2[32:33, :, :],
            in_=esd_sb[0:8, 0:R],
        )

        # ---- stage C: masked softmax + alpha @ h, four heads per pass ----
        zpool = ctx.enter_context(tc.tile_pool(name="zpool", bufs=1, space="PSUM"))
        opool = ctx.enter_context(tc.tile_pool(name="opool", bufs=1, space="PSUM"))
        lpool = ctx.enter_context(tc.tile_pool(name="lpool", bufs=2))
        ppool = ctx.enter_context(tc.tile_pool(name="ppool", bufs=2))
        npool = ctx.enter_context(tc.tile_pool(name="npool", bufs=2))

        G = 4
        for _rep in range(repeat):
            for hg in range(H // G):
                h0 = G * hg
                pout = opool.tile([F + 1, G * R], FP, tag="pout")
                for jt in range(JT):
                    pz = zpool.tile([128, G * R], FP, tag="pz")
                    sa = (2 * jt) % 4
                    sb = sa + 1
                    nc.gpsimd.dma_start(
                        out=dst_quad[1:34:32, sa, :],
                        in_=esd_sb[8 + h0:10 + h0, jt * 128:(jt + 1) * 128],
                    )
                    nc.gpsimd.dma_start(
                        out=dst_quad[1:34:32, sb, :],
                        in_=esd_sb[10 + h0:12 + h0, jt * 128:(jt + 1) * 128],
                    )
                    for hl in range(G):
                        b = 32 * (hl % 2)
                        s = sa if hl < 2 else sb
                        nc.tensor.matmul(
                            pz[:, hl * R:(hl + 1) * R],
                            lhsT=dst_quad[b:b + 2, s, :],
                            rhs=zsrc2[b:b + 2, h0 + hl, :],
                            start=True, stop=False, skip_group_check=True,
                        )
                    for hl in range(G):
                        nc.tensor.matmul(
                            pz[:, hl * R:(hl + 1) * R],
                            lhsT=identb_sb[:],
                            rhs=mask_sb[:, jt, :],
                            start=False, stop=True, skip_group_check=True,
                        )
                    zl = lpool.tile([128, G * R], FP, tag="zl")
                    nc.scalar.activation(out=zl[:], in_=pz[:], func=Act.Prelu, alpha=0.2)
                    pp = ppool.tile([128, G * R], FP, tag="pp")
                    nc.scalar.activation(out=pp[:], in_=zl[:], func=Act.Exp)
                    for hl in range(G):
                        sl = slice(hl * R, (hl + 1) * R)
                        nc.tensor.matmul(
                            pout[:, sl],
                            lhsT=haug_sb[:, jt, h0 + hl, :],
                            rhs=pp[:, sl],
                            start=(jt == 0), stop=(jt == JT - 1),
                            skip_group_check=True,
                        )
                # normalize: transpose chunks into bank-aligned slots of the
                # aliased pz scratch (matmul PSUM targets must be
                # bank-aligned), then batched reciprocal + scalar multiply.
                osb = npool.tile([F + 1, G * R], FP, tag="osb")
                nc.vector.tensor_copy(out=osb[:], in_=pout[:])
                for rnd in range(4):
                    pt = zpool.tile([128, G * R], FP, tag="pz")
                    for qq in range(4):
                        q = rnd * 4 + qq
                        nc.tensor.transpose(
                            pt[:, qq * R:qq * R + F + 1],
                            osb[:, q * 128:(q + 1) * 128],
                            identf_sb[0:F + 1, 0:F + 1],
                        )
                    ptv = pt[:].rearrange("p (q c) -> p q c", c=R)
                    rc = npool.tile([128, 4], FP, tag="rc")
                    nc.vector.reciprocal(rc[:], ptv[:, :, F])
                    for qq in range(4):
                        q = rnd * 4 + qq
                        hl, ic = q // 4, q % 4
                        nc.vector.tensor_scalar_mul(
                            outsb[:, ic, (h0 + hl) * F:(h0 + hl + 1) * F],
                            ptv[:, qq, 0:F],
                            rc[:, qq:qq + 1],
                        )

        nc.sync.dma_start(
            out_ap.rearrange("(ic p) f -> p ic f", p=128),
            outsb[:],
        )


def _host_prep(x, edge_index, W, a):
    x = np.asarray(x, np.float32)
    W = np.asarray(W, np.float32)
    a = np.asarray(a, np.float32)
    src = np.asarray(edge_index[0]).astype(np.int64)
    dst = np.asarray(edge_index[1]).astype(np.int64)

    A = np.zeros((H * F, 2 * H), np.float32)
    for h in range(H):
        A[h * F:(h + 1) * F, h] = a[h, :F]
        A[h * F:(h + 1) * F, 8 + h] = a[h, F:]
    wa = np.ascontiguousarray(W @ A)

    maskadd = np.full((NCORES, N, R), NEG, np.float32)
    c_of = src // R
    i_loc = src % R
    r = (dst - c_of * R) % N
    maskadd[c_of, r, i_loc] = 0.0
    idx = np.arange(R)
    maskadd[:, idx, idx] = 0.0
    maskadd = maskadd.astype(ml_dtypes.bfloat16)

    identb = np.eye(128, dtype=ml_dtypes.bfloat16)
    identf = np.eye(128, dtype=np.float32)
    onesh = np.ones((1, H, R), np.float32)

    in_maps = []
    for c in range(NCORES):
        xT_c = np.ascontiguousarray(np.roll(x, -c * R, axis=0).T)
        in_maps.append({
            "xT": xT_c,
            "w": W,
            "wa": wa,
            "maskadd": np.ascontiguousarray(maskadd[c]),
            "identb": identb,
            "identf": identf,
            "onesh": onesh,
        })
    return in_maps


_CACHED = {}


def _get_program(repeat=1):
    if repeat not in _CACHED:
        _CACHED[repeat] = _build_program(repeat)
    return _CACHED[repeat]


def kernel(x, edge_index, W, a, _repeat=1):
    nc = _get_program(_repeat)
    in_maps = _host_prep(x, edge_index, W, a)
    res = run_bass_kernel_spmd(nc, in_maps, core_ids=list(range(NCORES)))
    out = np.concatenate([res.results[c]["out"] for c in range(NCORES)], axis=0)
    return out.astype(np.float32)



# revision 2
# speedup vs baseline: 1.4027x; 1.4027x over previous
"""Trainium2 Bass kernel for CustomGATConv — v3 (dense, f16, DVE z-build).

Strategy (8-core SPMD, row-sharded attention):
  - Each core owns 512 destination rows i of the [4096, 4096, 8] attention
    tensor; inputs are node-rotated per core so every core's rows are
    [0:512) of its own rotated node order.
  - h = x @ W and (es, ed) = x @ WA are computed on device in fp16.
  - z[j, (jt, h, i)] = ed[j, h] + mask[j, i] + es[h, i] is built entirely on
    the Vector engine with stride-0 broadcast access patterns (two adds),
    batched over JB j-tiles per instruction — no PE work and no PSUM.
    The additive {0, -192} mask implements adjacency masking through the
    exp (exp(leaky(-192 + O(1))) == 0 in fp16).
  - leaky_relu as one scalar_tensor_tensor: max(0.2*z, z); exp on ScalarE.
  - alpha @ h accumulates [65, 2048] per 4-head group in PSUM via fp16
    matmuls against h augmented with a ones column (softmax denominator
    rides along as row 64).
  - Normalization: one DMA transpose of the [65, 2048] accumulator to
    [128, 16, 65], a batched reciprocal of the denominator column, and a
    single broadcast tensor_tensor multiply.
"""

import re

import numpy as np
import ml_dtypes

import bass_rust as br
import concourse.bass as bass
import concourse.tile as tile
from concourse import mybir
from concourse.bass_utils import run_bass_kernel_spmd

N = 4096
IN = 256
H = 8
F = 64
NCORES = 8
R = N // NCORES          # 512 destination rows per core
JT = N // 128            # 32 j-tiles
KC = IN // 128           # 2 contraction chunks for x @ W
JB = 4                   # j-tiles batched per elementwise instruction
NEG = -192.0             # additive mask value (exactly representable in f16)
FP = mybir.dt.float32
F16 = mybir.dt.float16
AP = bass.AP
Alu = mybir.AluOpType
Act = mybir.ActivationFunctionType


def _bcast(ap, dim, size):
    """Insert a stride-0 (broadcast) dim into an AP at position `dim`."""
    lay = [list(d) for d in ap.ap]
    lay.insert(dim, [0, size])
    return AP(ap.tensor, ap.offset, lay)


class _TileContext(tile.TileContext):
    """TileContext whose final drain splits its semaphore waits one per
    instruction — this walrus's CTRL_NO encoding only fits one sync wait."""

    def _drain_and_barrier(self, tick_clock, wait_clock):
        gc = tick_clock.global_clock
        vals = list(map(int, re.findall(r"\d+", repr(gc))))
        nonzero = [(i, t) for i, t in enumerate(vals) if t > 0]
        prev = br.VectorClock()
        partial = br.VectorClock()
        for i, t in nonzero:
            partial.require_at_least(i, t)
            inst = self.nc.sync.drain().ins
            wait_clock.add_sem_waits(
                inst,
                br.ScopedClock({None: partial.copy()}),
                br.ScopedClock({None: prev.copy()}),
            )
            prev = partial.copy()
        drain_inst = self.nc.sync.drain().ins
        wait_clock.add_sem_waits(
            drain_inst,
            br.ScopedClock({None: gc}),
            br.ScopedClock({None: prev.copy()}),
        )
        self.nc.all_engine_barrier()
        popped = self.nc._tile_sem_poison_stack.pop()
        assert popped is self._sem_poison
        self.nc.clear_and_free_semaphores(list(self.sems.allocated().values()))
        self.nc.all_engine_barrier()


def _split_excess_waits(nc, cap_compute=1, cap_nop=1):
    """This walrus encodes at most ~2 sync waits per compute instruction and
    1 per CTRL_NO (nop/drain).  Move excess waits onto injected same-engine
    nops placed immediately before the over-subscribed instruction."""
    n_split = 0
    for fn in nc.m.functions:
        for bb in fn.blocks:
            lst = bb.instructions
            i = 0
            while i < len(lst):
                inst = lst[i]
                si = inst.sync_info
                waits = list(si.on_wait) if si is not None else []
                is_ctrl = isinstance(inst, (mybir.InstNoOp, mybir.InstDrain))
                cap = cap_nop if is_ctrl else cap_compute
                if len(waits) > cap:
                    excess, keep = waits[:-cap], waits[-cap:]
                    for w in excess:
                        nop = mybir.InstNoOp(name=f"waitsplit-{nc.next_id()}")
                        nop.engine = inst.engine
                        nop.sync_info = br.SyncInfo(on_wait=[w], on_update=[])
                        lst.insert(i, nop)
                        i += 1
                        n_split += 1
                    inst.sync_info = br.SyncInfo(
                        on_wait=keep, on_update=list(si.on_update)
                    )
                i += 1
    return n_split


def _build_program(repeat=1):
    nc = bass.Bass("TRN2", target_bir_lowering=False, debug=False)
    ap = {}
    ap["xT"] = nc.dram_tensor("xT", [128, KC, N], F16, kind="ExternalInput").ap()
    ap["w"] = nc.dram_tensor("w", [128, KC, H * F], F16, kind="ExternalInput").ap()
    ap["wa"] = nc.dram_tensor("wa", [128, KC, 2 * H], F16, kind="ExternalInput").ap()
    ap["mask"] = nc.dram_tensor("mask", [128, JT * R], F16, kind="ExternalInput").ap()
    ap["ind"] = nc.dram_tensor("ind", [8, H * 128], F16, kind="ExternalInput").ap()
    out_ap = nc.dram_tensor("out", [R, H * F], FP, kind="ExternalOutput").ap()

    with _TileContext(nc) as tc:
        _emit(tc, nc, ap, out_ap, repeat)
    _split_excess_waits(nc)
    return nc


def _emit(tc, nc, ap, out_ap, repeat):
    from contextlib import ExitStack

    with ExitStack() as ctx:
        singles = ctx.enter_context(tc.tile_pool(name="singles", bufs=1))

        # ---- persistent tiles ----
        mask_sb = singles.tile([128, JT, R], F16)
        nc.sync.dma_start(mask_sb[:], ap["mask"].rearrange("p (jt i) -> p jt i", jt=JT))
        haug = singles.tile([128, JT, H, F + 1], F16)
        nc.vector.memset(haug[:, :, :, F:F + 1], 1.0)
        esd16 = singles.tile([16, N], F16)
        # esd, node-major: edjT[p, jt, 0:8] = es, [p, jt, 8:16] = ed for node
        # jt*128+p — one DMA transpose of esd16.
        edjT = singles.tile([128, JT, 16], F16)
        esrep = singles.tile([128, 2, 4 * R], F16)
        # ind[k, h*128 + p] = 1 if k == h else 0 — lhsT blocks for replicating
        # es row h to all 128 partitions (base partition 0, K=8).
        ind = singles.tile([8, H * 128], F16)
        nc.sync.dma_start(ind[:], ap["ind"])
        outsb = singles.tile([128, 4, H * F], FP)
        # DMA-transpose staging: 80 partitions (xbar needs a multiple of 16);
        # rows 65..79 are pad, initialized once.
        osb2 = [singles.tile([80, 4 * R], F16, name=f"osb{i}") for i in range(2)]
        for o in osb2:
            nc.vector.memset(o[:], 1.0)

        # ---- stage B: h = x @ W (node-major), esd = (x @ WA)^T, tables ----
        with tc.tile_pool(name="bigin", bufs=1) as bigin, \
             tc.tile_pool(name="hpsum", bufs=2, space="PSUM") as hpsum:
            xT_sb = bigin.tile([128, KC, N], F16)
            nc.sync.dma_start(xT_sb[:], ap["xT"])
            w_sb = bigin.tile([128, KC, H * F], F16)
            nc.sync.dma_start(w_sb[:], ap["w"])
            wa_sb = bigin.tile([128, KC, 2 * H], F16)
            nc.sync.dma_start(wa_sb[:], ap["wa"])

            for m in range(JT):
                ph = hpsum.tile([128, H * F], FP, tag="ph")
                for k in range(KC):
                    nc.tensor.matmul(
                        ph[:],
                        lhsT=xT_sb[:, k, m * 128:(m + 1) * 128],
                        rhs=w_sb[:, k, :],
                        start=(k == 0),
                        stop=(k == KC - 1),
                    )
                nc.vector.tensor_copy(
                    out=haug[:, m, :, 0:F],
                    in_=ph[:].rearrange("p (h f) -> p h f", h=H),
                )
            for q in range(8):
                pe = hpsum.tile([16, R], FP, tag="pe")
                for k in range(KC):
                    nc.tensor.matmul(
                        pe[:],
                        lhsT=wa_sb[:, k, :],
                        rhs=xT_sb[:, k, q * R:(q + 1) * R],
                        start=(k == 0),
                        stop=(k == KC - 1),
                    )
                nc.vector.tensor_copy(out=esd16[:, q * R:(q + 1) * R], in_=pe[:])

            nc.sync.dma_start_transpose(edjT[:], esd16[:])
            # esrep[p, hg, hl*R + i] = es[hg*4+hl, i]: replicate row h of esd16
            # to all partitions with a K=8 indicator matmul (base partition 0).
            for hg in range(2):
                pr = hpsum.tile([128, 4 * R], FP, tag="pr", bufs=1)
                for hl in range(4):
                    h = hg * 4 + hl
                    nc.tensor.matmul(
                        pr[:, hl * R:(hl + 1) * R],
                        lhsT=ind[:, h * 128:(h + 1) * 128],
                        rhs=esd16[0:8, 0:R],
                        start=True, stop=True, skip_group_check=True,
                    )
                nc.vector.tensor_copy(out=esrep[:, hg, :], in_=pr[:])

        # ---- stage C: masked softmax + alpha @ h ----
        opool = ctx.enter_context(tc.tile_pool(name="opool", bufs=2, space="PSUM"))
        tpool = ctx.enter_context(tc.tile_pool(name="tpool", bufs=2))
        upool = ctx.enter_context(tc.tile_pool(name="upool", bufs=2))
        ppool = ctx.enter_context(tc.tile_pool(name="ppool", bufs=2))
        npool = ctx.enter_context(tc.tile_pool(name="npool", bufs=2))

        for _rep in range(repeat):
            for hg in range(2):
                pout = opool.tile([F + 1, 4 * R], FP, tag="pout")
                for blk in range(JT // JB):
                    jt0 = blk * JB
                    # t = ed (bcast over i) + mask (bcast over heads)
                    t = tpool.tile([128, JB, 4, R], F16, tag="t")
                    ed_b = _bcast(
                        edjT[:, jt0:jt0 + JB, 8 + hg * 4:8 + hg * 4 + 4], 3, R)
                    mask_b = _bcast(mask_sb[:, jt0:jt0 + JB, :], 2, 4)
                    nc.vector.tensor_tensor(out=t[:], in0=ed_b, in1=mask_b, op=Alu.add)
                    # u = t + es (bcast over j-tiles)
                    u = upool.tile([128, JB, 4, R], F16, tag="u")
                    es_b = _bcast(esrep[:, hg, :].rearrange("p (hl i) -> p hl i", hl=4), 1, JB)
                    nc.vector.tensor_tensor(out=u[:], in0=t[:], in1=es_b, op=Alu.add)
                    # u = leaky_relu(u) = max(0.2*u, u), in place
                    nc.vector.scalar_tensor_tensor(
                        out=u[:], in0=u[:], scalar=0.2, in1=u[:],
                        op0=Alu.mult, op1=Alu.max)
                    # pp = exp(u)
                    pp = ppool.tile([128, JB, 4, R], F16, tag="pp")
                    nc.scalar.activation(out=pp[:], in_=u[:], func=Act.Exp)
                    for q in range(JB):
                        jt = jt0 + q
                        for hl in range(4):
                            nc.tensor.matmul(
                                pout[:, hl * R:(hl + 1) * R],
                                lhsT=haug[:, jt, hg * 4 + hl, :],
                                rhs=pp[:, q, hl, :],
                                start=(jt == 0), stop=(jt == JT - 1),
                                skip_group_check=True,
                            )
                # normalize: transpose [80, 2048] -> [128, 16, 80] via DMA,
                # batched reciprocal of the denominator column, one broadcast
                # multiply into outsb.
                osb = osb2[hg]
                nc.vector.tensor_copy(out=osb[0:F + 1, :], in_=pout[:])
                ptt = npool.tile([128, 16, 80], F16, tag="ptt")
                nc.sync.dma_start_transpose(ptt[:], osb[:])
                rc = npool.tile([128, 16], FP, tag="rc")
                nc.vector.reciprocal(rc[:], ptt[:, :, F])
                out_view = outsb[:, :, hg * 4 * F:(hg + 1) * 4 * F].rearrange(
                    "p ic (hl f) -> p hl ic f", f=F)
                rc_b = _bcast(rc[:].rearrange("p (hl ic) -> p hl ic", hl=4), 3, F)
                nc.vector.tensor_tensor(
                    out=out_view,
                    in0=ptt[:, :, 0:F].rearrange("p (hl ic) f -> p hl ic f", hl=4),
                    in1=rc_b, op=Alu.mult)

        nc.sync.dma_start(
            out_ap.rearrange("(ic p) f -> p ic f", p=128),
            outsb[:],
        )


def _host_prep(x, edge_index, W, a):
    x = np.asarray(x, np.float32)
    W = np.asarray(W, np.float32)
    a = np.asarray(a, np.float32)
    src = np.asarray(edge_index[0]).astype(np.int64)
    dst = np.asarray(edge_index[1]).astype(np.int64)

    # wa[:, 0:8] = W_h @ a_src_h ; wa[:, 8:16] = W_h @ a_dst_h
    W3 = W.reshape(IN, H, F)
    wa = np.concatenate([
        np.einsum("ihf,hf->ih", W3, a[:, :F]),
        np.einsum("ihf,hf->ih", W3, a[:, F:]),
    ], axis=1)

    # Additive mask in the rotated frame: rows i local to each core, columns
    # j rotated so each core's own rows come first.
    M = np.full((NCORES, N, R), NEG, np.float32)
    c_of = src // R
    i_loc = src % R
    r = (dst - c_of * R) % N
    M[c_of, r, i_loc] = 0.0
    idx = np.arange(R)
    M[:, idx, idx] = 0.0
    M16 = M.astype(np.float16)

    ind = np.zeros((8, H * 128), np.float16)
    for h in range(H):
        ind[h, h * 128:(h + 1) * 128] = 1.0

    w16 = np.ascontiguousarray(
        W.reshape(KC, 128, H * F).transpose(1, 0, 2)).astype(np.float16)
    wa16 = np.ascontiguousarray(
        wa.reshape(KC, 128, 2 * H).transpose(1, 0, 2)).astype(np.float16)

    in_maps = []
    for c in range(NCORES):
        x_rot = np.roll(x, -c * R, axis=0)
        xT16 = np.ascontiguousarray(
            x_rot.T.reshape(KC, 128, N).transpose(1, 0, 2)).astype(np.float16)
        mask_c = np.ascontiguousarray(
            M16[c].reshape(JT, 128, R).transpose(1, 0, 2).reshape(128, JT * R))
        in_maps.append({
            "xT": xT16,
            "w": w16,
            "wa": wa16,
            "mask": mask_c,
            "ind": ind,
        })
    return in_maps


_CACHED = {}


def _get_program(repeat=1):
    if repeat not in _CACHED:
        _CACHED[repeat] = _build_program(repeat)
    return _CACHED[repeat]


def kernel(x, edge_index, W, a, _repeat=1):
    nc = _get_program(_repeat)
    in_maps = _host_prep(x, edge_index, W, a)
    res = run_bass_kernel_spmd(nc, in_maps, core_ids=list(range(NCORES)))
    out = np.concatenate([res.results[c]["out"] for c in range(NCORES)], axis=0)
    return out.astype(np.float32)
